# revision 19
# baseline (speedup 1.0000x reference)
"""3-layer GAT forward on 8 Trainium2 cores — v3 (dma_gather edition).

Structure (per layer):
  - Nodes sorted by in-degree and dealt round-robin to 8 cores, then packed
    into 98 blocks of 128 consecutive (uniform-degree) nodes per core; the
    node's slot is its position in the block. Degree-sorted dealing makes the
    per-(block, slab) tile structure near-identical across cores, so one SPMD
    program (padded to the max across cores) serves all 8.
  - Gather table rows are 256 B: [W@h bf16 x64 | el f32 x4 | pad]. Each core
    computes its own rows (phase A), AllGathers the table, then gathers
    per-edge source rows with dma_gather (int16 indices -> the table is
    addressed in 4 slabs of 2 cores, 25088 rows each). Edges are grouped by
    (block-group, slab); within a (block, slab) segment they are sorted by
    destination slot and split into 128 balanced chunks -> partition p's
    edges span only a few consecutive slots.
  - er (attention right-term) per edge: one plain indirect DMA per block
    reads, for each partition, k consecutive er rows starting at its chunk's
    min slot; a small one-hot (q = slot - minslot) selects per-edge er on DVE.
  - ee = exp(leaky_relu(el+er)); V = [ee*feat | ee] bf16; one-hot O over dst
    slots (bf16); per-tile matmul accumulates [sum(ee*feat) | sum(ee)] in
    PSUM; softmax divide per node afterwards. Layers 0/1 transposed
    ([channel, node]) with BN stats AllReduced; layer 2 node-major.
"""

import numpy as np
import ml_dtypes

import concourse.bass as bass
import concourse.bacc as bacc
import concourse.tile as tile
from concourse import mybir, bass_utils, library_config

F32 = mybir.dt.float32
I32 = mybir.dt.int32
I16 = mybir.dt.int16
I8 = mybir.dt.int8
BF16 = mybir.dt.bfloat16
BF = ml_dtypes.bfloat16

N, E, NC = 100000, 1600000, 8
IN, HID, HEADS, OUTC = 128, 16, 4, 40
F = HEADS * HID            # 64
F2 = HEADS * OUTC          # 160
P = 128
NPC = N // NC              # 12500
B = (NPC + P - 1) // P     # 98
NOWN = B * P               # 12544
NPAD = NC * NOWN
NSLAB = 4
SLABR = 2 * NOWN           # 25088 rows per index slab (< 2^15)
ELEM = 128                 # bf16 units per table row (256 B)
ELEM2 = 256                # layer-2 row (512 B)
NEG, EPS = 0.2, 1e-5
TB = 48                    # tile budget per block-group
GBMAX = 8                  # max blocks per group
SLABMAX = 48               # max tiles of one (group, slab)
KMAX = 64                  # max er slot-span


# ---------------------------------------------------------------- host prep

class Prep:
    pass


def preprocess(src, dst):
    """Graph preprocessing. Returns a Prep with global structure + per-core
    index arrays."""
    src = np.asarray(src, np.int64)
    dst = np.asarray(dst, np.int64)
    pr = Prep()
    deg = np.bincount(dst, minlength=N)
    order = np.argsort(-deg, kind="stable")
    core_of = np.empty(N, np.int64)
    pos = np.empty(N, np.int64)
    core_of[order] = np.arange(N) % NC
    pos[order] = np.arange(N) // NC
    pr.core_of, pr.pos = core_of, pos

    b_e = pos[dst] // P
    s_e = pos[dst] % P
    r_e = core_of[dst]
    c4_e = core_of[src] // 2
    srow_e = (core_of[src] % 2) * NOWN + pos[src]

    # per (core, block, slab) segment sizes
    key = (r_e * B + b_e) * NSLAB + c4_e
    cnt = np.bincount(key, minlength=NC * B * NSLAB).reshape(NC, B, NSLAB)
    S = np.ceil(cnt.max(axis=0) / P).astype(np.int64)      # [B, NSLAB]
    S = np.maximum(S, 1)
    Tb = S.sum(axis=1)                                      # tiles per block
    assert Tb.max() <= TB

    # block groups under tile budget TB (and <= GBMAX blocks)
    groups = []
    b0 = 0
    while b0 < B:
        b1, tot = b0, 0
        while b1 < B and b1 - b0 < GBMAX and tot + Tb[b1] <= TB:
            tot += Tb[b1]
            b1 += 1
        groups.append((b0, b1))
        b0 = b1
    pr.groups = groups
    pr.S, pr.Tb = S, Tb

    # column layouts: per (group, slab, block, tile)
    # tile-column order (for colv/q/G): g-major, then slab, then block, tile
    tile_off = {}      # (g, c4, b) -> tile col offset (global)
    ioff = {}          # (g, c4) -> idx16 column offset, n_idx
    goff = []          # group -> (tile col offset, total tiles)
    tcol = 0
    icol = 0
    for g, (ga, gb) in enumerate(groups):
        gt0 = tcol
        for c4 in range(NSLAB):
            n_idx = int(S[ga:gb, c4].sum()) * P
            ioff[(g, c4)] = (icol, n_idx)
            icol += n_idx // 16
            for b in range(ga, gb):
                tile_off[(g, c4, b)] = tcol
                tcol += int(S[b, c4])
        goff.append((gt0, tcol - gt0))
    TOT_TILES = tcol
    pr.tile_off, pr.ioff, pr.goff = tile_off, ioff, goff
    pr.TOT_TILES, pr.TOTC16 = TOT_TILES, icol

    # per-core arrays
    sortk = np.lexsort((s_e, key))
    eid_sorted = np.arange(E)[sortk]
    starts = np.concatenate([[0], np.cumsum(cnt.reshape(-1))])

    pr.idx16 = np.zeros((NC, 128, icol), np.int16)
    pr.colv = np.full((NC, P, TOT_TILES), -1, np.int8)
    pr.qv = np.full((NC, P, TOT_TILES), -1, np.int8)
    pr.minsl = np.zeros((NC, P, B), np.int32)
    kg = np.ones(len(groups), np.int64)

    for r in range(NC):
        mnsl = np.full((B, P), 10000, np.int64)
        mxsl = np.full((B, P), -1, np.int64)
        seg_cache = {}
        for b in range(B):
            for c4 in range(NSLAB):
                seg = (r * B + b) * NSLAB + c4
                e0, e1 = starts[seg], starts[seg + 1]
                eids = eid_sorted[e0:e1]          # slot-sorted
                n = e1 - e0
                base, rem = divmod(int(n), P)
                sizes = np.full(P, base, np.int64)
                sizes[:rem] += 1
                cstart = np.concatenate([[0], np.cumsum(sizes)])
                seg_cache[(b, c4)] = (eids, sizes, cstart)
                if n:
                    sl = s_e[eids]
                    pidx = np.searchsorted(cstart, np.arange(int(n)),
                                           side="right") - 1
                    np.minimum.at(mnsl[b], pidx, sl)
                    np.maximum.at(mxsl[b], pidx, sl)
        mnsl = np.where(mxsl >= 0, mnsl, 0)
        span = np.where(mxsl >= 0, mxsl - mnsl + 1, 1)
        pr.minsl[r] = (np.arange(B)[:, None] * P + mnsl).T.astype(np.int32)
        for g, (ga, gb) in enumerate(groups):
            kg[g] = max(kg[g], int(span[ga:gb].max()))
        for g, (ga, gb) in enumerate(groups):
            for c4 in range(NSLAB):
                ic0, n_idx = pr.ioff[(g, c4)]
                jt = 0   # tile index within this instruction
                for b in range(ga, gb):
                    eids, sizes, cstart = seg_cache[(b, c4)]
                    tc0 = pr.tile_off[(g, c4, b)]
                    sl = s_e[eids]
                    sr = srow_e[eids]
                    for t in range(int(S[b, c4])):
                        # partition p takes chunk p's t-th edge
                        pmask = sizes > t
                        ps = np.nonzero(pmask)[0]
                        ei = cstart[ps] + t
                        col = tc0 + t
                        pr.colv[r, ps, col] = sl[ei].astype(np.int8)
                        pr.qv[r, ps, col] = (sl[ei] - mnsl[b, ps]).astype(
                            np.int8)
                        # idx16 for j = (jt*128 + p)
                        j = (jt * P + ps)
                        v = sr[ei].astype(np.int16)
                        pr.idx16[r, (j % 16), ic0 + j // 16] = v
                        jt += 1
        # replicate idx rows across the 8 16-partition bands
        pr.idx16[r] = np.tile(pr.idx16[r, :16], (8, 1))
    pr.kg = [int(k) for k in kg]
    pr.KM = max(pr.kg)
    assert pr.KM <= KMAX, pr.kg
    pr.SM = max(int(S[ga:gb, c4].sum())
                for (ga, gb) in groups for c4 in range(NSLAB))
    assert pr.SM <= SLABMAX
    return pr


def _fold(W, a, dph):
    return np.einsum("khd,hd->kh", W.reshape(W.shape[0], HEADS, dph),
                     a).astype(np.float32)


# ---------------------------------------------------------------- program

def build_layer_program(pr, layer):
    fo = F2 if layer == 2 else F
    fin = IN if layer == 0 else F
    elem = ELEM2 if layer == 2 else ELEM
    fo4 = fo + 4
    nc = bacc.Bacc("TRN2", target_bir_lowering=False, debug=False,
                   num_devices=NC)
    S, groups, kg = pr.S, pr.groups, pr.kg

    def inp(name, shape, dt=F32):
        return nc.dram_tensor(name, shape, dt, kind="ExternalInput").ap()

    x_in = inp("x_in", [fin, NOWN], BF16)
    idx16_i = inp("idx16", [128, pr.TOTC16], I16)
    colv_i = inp("colv", [P, pr.TOT_TILES], I8)
    qv_i = inp("qv", [P, pr.TOT_TILES], I8)
    minsl_i = inp("minsl", [P, B], I32)
    Wext_i = inp("Wext", [fin, fo + 8], BF16)
    iota_i = inp("iota", [P, P], BF16)
    if layer < 2:
        gamma_i = inp("gamma", [F, 1])
        beta_i = inp("beta", [F, 1])
        bsel_i = inp("bsel", [HEADS, F])
        y_out = nc.dram_tensor("y_out", [F, NOWN], BF16,
                               kind="ExternalOutput").ap()
    if layer == 0:
        resW_i = inp("resW", [IN, F], BF16)
    if layer == 2:
        resW_i = inp("resW", [F, F2], BF16)
        b2bc_i = inp("b2bc", [P, F2])
        out = nc.dram_tensor("out", [NOWN, F2], F32,
                             kind="ExternalOutput").ap()

    rg = [list(range(NC))]

    with tile.TileContext(nc) as tc:
        with (
            tc.tile_pool(name="big", bufs=1) as bigp,
            tc.tile_pool(name="const", bufs=1) as cons,
            tc.tile_pool(name="gt", bufs=2) as gtp,
            tc.tile_pool(name="wk", bufs=2) as wk,
            tc.tile_pool(name="sm", bufs=3 if layer < 2 else 2) as sm,
            tc.tile_pool(name="ps", bufs=2, space="PSUM") as ps,
            tc.tile_pool(name="dram", bufs=1, space="DRAM") as dr,
        ):
            nc.gpsimd.load_library(library_config.mlp)

            tbl_own = dr.tile([NOWN, elem], BF16)
            tbl_full = dr.tile([NPAD, elem], BF16, addr_space="Shared")
            er_dram = dr.tile([NOWN + 2 * P, 4], F32)
            if layer < 2:
                stats_in = dr.tile([F, 2], F32)
                stats_out = dr.tile([F, 2], F32, addr_space="Shared")

            xT = bigp.tile([fin, NOWN], BF16, tag="xT")
            nc.sync.dma_start(out=xT[:], in_=x_in[:])
            if layer < 2:
                yT = bigp.tile([F, NOWN], BF16, tag="yT")
            colv_all = bigp.tile([P, pr.TOT_TILES], I8, tag="colv")
            qv_all = bigp.tile([P, pr.TOT_TILES], I8, tag="qv")
            minsl = bigp.tile([P, B], I32, tag="minsl")
            nc.sync.dma_start(out=colv_all[:], in_=colv_i[:])
            nc.sync.dma_start(out=qv_all[:], in_=qv_i[:])
            nc.sync.dma_start(out=minsl[:], in_=minsl_i[:])

            Wext = cons.tile([fin, fo + 8], BF16, tag="Wext")
            nc.sync.dma_start(out=Wext[:], in_=Wext_i[:])
            iota_b = cons.tile([P, P], BF16, tag="iota")
            nc.sync.dma_start(out=iota_b[:], in_=iota_i[:])
            if layer < 2:
                bsel = cons.tile([HEADS, F], F32, tag="bsel")
                gam = cons.tile([F, 1], F32, tag="gam")
                bet = cons.tile([F, 1], F32, tag="bet")
                nc.sync.dma_start(out=bsel[:], in_=bsel_i[:])
                nc.sync.dma_start(out=gam[:], in_=gamma_i[:])
                nc.sync.dma_start(out=bet[:], in_=beta_i[:])
                stats = cons.tile([F, 2], F32, tag="stats")
                nc.vector.memset(stats[:], 0.0)
            if layer == 0:
                rW = cons.tile([IN, F], BF16, tag="rW")
                nc.sync.dma_start(out=rW[:], in_=resW_i[:])
            if layer == 2:
                rW = cons.tile([F, F2], BF16, tag="rW")
                nc.sync.dma_start(out=rW[:], in_=resW_i[:])
                b2bc = cons.tile([P, F2], F32, tag="b2bc")
                nc.sync.dma_start(out=b2bc[:], in_=b2bc_i[:])
            zero4 = cons.tile([P, 4], F32, tag="zero4")
            nc.vector.memset(zero4[:], 0.0)

            # ---- phase A: table rows + er + AllGather ----
            CH = 7
            for c0 in range(0, B, CH):
                stg = wk.tile([P, CH * elem], BF16, tag="stg")
                stg_e = wk.tile([P, CH * 4], F32, tag="stg_e")
                nc.vector.memset(stg[:], 0.0)
                for ci in range(CH):
                    b = c0 + ci
                    tp = ps.tile([P, fo + 8], F32, tag="tp")
                    nc.tensor.matmul(out=tp[:], lhsT=xT[:, b * P:(b + 1) * P],
                                     rhs=Wext[:], start=True, stop=True)
                    nc.vector.tensor_copy(out=stg[:, ci * elem:ci * elem + fo],
                                          in_=tp[:, :fo])
                    nc.vector.tensor_copy(
                        out=stg[:, ci * elem + fo:ci * elem + fo + 8]
                        .bitcast(F32),
                        in_=tp[:, fo:fo + 4])
                    nc.vector.tensor_copy(out=stg_e[:, ci * 4:(ci + 1) * 4],
                                          in_=tp[:, fo + 4:fo + 8])
                nc.sync.dma_start(
                    out=tbl_own[c0 * P:(c0 + CH) * P, :].rearrange(
                        "(c p) f -> p c f", c=CH),
                    in_=stg[:].rearrange("p (c f) -> p c f", c=CH))
                nc.sync.dma_start(
                    out=er_dram[c0 * P:(c0 + CH) * P, :].rearrange(
                        "(c p) f -> p c f", c=CH),
                    in_=stg_e[:].rearrange("p (c f) -> p c f", c=CH))
            nc.sync.dma_start(out=er_dram[NOWN:NOWN + P, :], in_=zero4[:])
            nc.sync.dma_start(out=er_dram[NOWN + P:NOWN + 2 * P, :],
                              in_=zero4[:])
            nc.gpsimd.collective_compute(
                "AllGather", mybir.AluOpType.bypass, replica_groups=rg,
                ins=[tbl_own[:].opt()], outs=[tbl_full[:].opt()])
            tc.strict_bb_all_engine_barrier()

            # ---- phase B ----
            KM = pr.KM
            SMX = pr.SM
            for g, (ga, gb) in enumerate(groups):
                k = kg[g]
                nb = gb - ga
                gt0, gtn = pr.goff[g]
                ic_a = pr.ioff[(g, 0)][0]
                ic_b = pr.ioff[(g, 3)][0] + pr.ioff[(g, 3)][1] // 16
                idxt = wk.tile([128, TB * 8], I16, tag="idxt")
                nc.sync.dma_start(out=idxt[:, :ic_b - ic_a],
                                  in_=idx16_i[:, ic_a:ic_b])

                G = gtp.tile([P, TB * elem], BF16, tag="G")
                for c4 in range(NSLAB):
                    ic0, n_idx = pr.ioff[(g, c4)]
                    t0 = pr.tile_off[(g, c4, ga)] - gt0
                    nt = n_idx // P
                    # SWDGE ring holds ~1024 descriptors; larger single
                    # gathers deadlock the Q7 descriptor generator.
                    for q0 in range(0, nt, 8):
                        nq = min(8, nt - q0)
                        nc.gpsimd.dma_gather(
                            G[:, (t0 + q0) * elem:(t0 + q0 + nq) * elem]
                            .rearrange("p (t e) -> p t e", e=elem),
                            tbl_full[c4 * SLABR:(c4 + 1) * SLABR, :],
                            idxt[:, ic0 - ic_a + q0 * 8:
                                 ic0 - ic_a + (q0 + nq) * 8],
                            nq * P, nq * P, elem)

                # er rows per block: one [P, k*4] contiguous-run gather each
                erg = wk.tile([P, GBMAX * KM * 4], F32, tag="erg")
                for b in range(ga, gb):
                    br = b - ga
                    nc.gpsimd.indirect_dma_start(
                        out=erg[:, br * KM * 4:br * KM * 4 + k * 4],
                        out_offset=None, in_=er_dram[:],
                        in_offset=bass.IndirectOffsetOnAxis(
                            ap=minsl[:, b:b + 1], axis=0))
                ergb = wk.tile([P, GBMAX * KM * 4], BF16, tag="ergb")
                nc.vector.tensor_copy(out=ergb[:, :nb * KM * 4],
                                      in_=erg[:, :nb * KM * 4])

                V = gtp.tile([P, TB * fo4], BF16, tag="V")
                O = gtp.tile([P, TB * P], BF16, tag="O")
                for c4 in range(NSLAB):
                    sgc = int(S[ga:gb, c4].sum())
                    gcol = pr.tile_off[(g, c4, ga)]          # global col
                    t0 = gcol - gt0                          # in group tile
                    colb = sm.tile([P, SMX], BF16, tag="colb")
                    qb = sm.tile([P, SMX], BF16, tag="qb")
                    nc.vector.tensor_copy(
                        out=colb[:, :sgc], in_=colv_all[:, gcol:gcol + sgc])
                    nc.vector.tensor_copy(
                        out=qb[:, :sgc], in_=qv_all[:, gcol:gcol + sgc])
                    Q = sm.tile([P, SMX * KM], BF16, tag="Q")
                    Q3 = Q[:, :sgc * k].rearrange("p (s k) -> p s k", k=k)
                    nc.vector.tensor_tensor(
                        out=Q3,
                        in0=qb[:, :sgc].unsqueeze(2).to_broadcast(
                            [P, sgc, k]),
                        in1=iota_b[:, :k].unsqueeze(1).to_broadcast(
                            [P, sgc, k]),
                        op=mybir.AluOpType.is_equal)
                    tmp = sm.tile([P, SMX * 4 * KM], BF16, tag="tmp")
                    tmp4 = tmp[:, :sgc * 4 * k].rearrange(
                        "p (s h k) -> p s h k", h=4, k=k)
                    for b in range(ga, gb):
                        sb = int(S[b, c4])
                        br = b - ga
                        bo = pr.tile_off[(g, c4, b)] - gcol
                        nc.vector.tensor_tensor(
                            out=tmp4[:, bo:bo + sb],
                            in0=Q3[:, bo:bo + sb].unsqueeze(2).to_broadcast(
                                [P, sb, 4, k]),
                            in1=ergb[:, br * KM * 4:br * KM * 4 + k * 4]
                            .rearrange("p (k h) -> p h k", h=4).unsqueeze(1)
                            .to_broadcast([P, sb, 4, k]),
                            op=mybir.AluOpType.mult)
                    ere = sm.tile([P, SMX * 4], F32, tag="ere")
                    nc.vector.reduce_sum(
                        out=ere[:, :sgc * 4],
                        in_=tmp[:, :sgc * 4 * k].rearrange(
                            "p (x k) -> p x k", k=k),
                        axis=mybir.AxisListType.X)

                    G3 = G[:, t0 * elem:(t0 + sgc) * elem].rearrange(
                        "p (t e) -> p t e", e=elem)
                    el_view = G3[:, :, fo:fo + 8].bitcast(F32)
                    ee = sm.tile([P, SMX * 4], F32, tag="ee")
                    ee3 = ee[:, :sgc * 4].rearrange("p (s h) -> p s h", h=4)
                    nc.vector.tensor_tensor(
                        out=ee3, in0=el_view,
                        in1=ere[:, :sgc * 4].rearrange("p (s h) -> p s h",
                                                       h=4),
                        op=mybir.AluOpType.add)
                    nc.vector.scalar_tensor_tensor(
                        out=ee[:, :sgc * 4], in0=ee[:, :sgc * 4], scalar=NEG,
                        in1=ee[:, :sgc * 4], op0=mybir.AluOpType.mult,
                        op1=mybir.AluOpType.max)
                    eeb = sm.tile([P, SMX * 4], BF16, tag="eeb")
                    nc.scalar.activation(out=eeb[:, :sgc * 4],
                                         in_=ee[:, :sgc * 4],
                                         func=mybir.ActivationFunctionType.Exp)
                    eeb3 = eeb[:, :sgc * 4].rearrange("p (s h) -> p s h", h=4)

                    V3 = V[:, t0 * fo4:(t0 + sgc) * fo4].rearrange(
                        "p (s f) -> p s f", f=fo4)
                    dph = OUTC if layer == 2 else HID
                    nc.vector.tensor_tensor(
                        out=V3[:, :, :fo].rearrange("p s (h d) -> p s h d",
                                                    h=HEADS),
                        in0=G3[:, :, :fo].rearrange("p s (h d) -> p s h d",
                                                    h=HEADS),
                        in1=eeb3.unsqueeze(3).to_broadcast(
                            [P, sgc, HEADS, dph]),
                        op=mybir.AluOpType.mult)
                    nc.vector.tensor_copy(out=V3[:, :, fo:fo4], in_=eeb3)

                    O3 = O[:, t0 * P:(t0 + sgc) * P].rearrange(
                        "p (s c) -> p s c", c=P)
                    nc.vector.tensor_tensor(
                        out=O3,
                        in0=iota_b[:].unsqueeze(1).to_broadcast([P, sgc, P]),
                        in1=colb[:, :sgc].unsqueeze(2).to_broadcast(
                            [P, sgc, P]),
                        op=mybir.AluOpType.is_equal)

                # matmuls + per-node epilogue, block-major
                for b in range(ga, gb):
                    ntile = int(pr.Tb[b])
                    if layer < 2:
                        acc = ps.tile([fo4, P], F32, tag="acc")
                    else:
                        acc = ps.tile([P, fo4], F32, tag="acc")
                    it = 0
                    for c4 in range(NSLAB):
                        for t in range(int(S[b, c4])):
                            j = pr.tile_off[(g, c4, b)] - gt0 + t
                            if layer < 2:
                                nc.tensor.matmul(
                                    out=acc[:],
                                    lhsT=V[:, j * fo4:(j + 1) * fo4],
                                    rhs=O[:, j * P:(j + 1) * P],
                                    start=(it == 0), stop=(it == ntile - 1))
                            else:
                                nc.tensor.matmul(
                                    out=acc[:],
                                    lhsT=O[:, j * P:(j + 1) * P],
                                    rhs=V[:, j * fo4:(j + 1) * fo4],
                                    start=(it == 0), stop=(it == ntile - 1))
                            it += 1

                    if layer < 2:
                        dmax = sm.tile([HEADS, P], F32, tag="dmax")
                        nc.vector.tensor_scalar_max(out=dmax[:],
                                                    in0=acc[F:F + 4, :],
                                                    scalar1=1e-16)
                        rec = sm.tile([HEADS, P], F32, tag="rec")
                        nc.vector.reciprocal(out=rec[:], in_=dmax[:])
                        recb_ps = ps.tile([F, P], F32, tag="aux")
                        nc.tensor.matmul(out=recb_ps[:], lhsT=bsel[:],
                                         rhs=rec[:], start=True, stop=True)
                        recb = sm.tile([F, P], F32, tag="recb")
                        nc.vector.tensor_copy(out=recb[:], in_=recb_ps[:])
                        hsl = yT[:, b * P:(b + 1) * P]
                        hf = sm.tile([F, P], F32, tag="hf")
                        nc.vector.tensor_tensor(out=hf[:], in0=acc[:F, :],
                                                in1=recb[:],
                                                op=mybir.AluOpType.mult)
                        if layer == 0:
                            res_ps = ps.tile([F, P], F32, tag="aux")
                            nc.tensor.matmul(out=res_ps[:], lhsT=rW[:],
                                             rhs=xT[:, b * P:(b + 1) * P],
                                             start=True, stop=True)
                            nc.vector.tensor_tensor(out=hsl, in0=hf[:],
                                                    in1=res_ps[:],
                                                    op=mybir.AluOpType.add)
                        else:
                            hold = xT[:, b * P:(b + 1) * P]
                            nc.vector.tensor_tensor(out=hf[:], in0=hf[:],
                                                    in1=hold,
                                                    op=mybir.AluOpType.add)
                            nc.vector.tensor_tensor(out=hsl, in0=hf[:],
                                                    in1=hold,
                                                    op=mybir.AluOpType.add)
                        red = sm.tile([F, 1], F32, tag="red")
                        nc.vector.reduce_sum(out=red[:], in_=hsl,
                                             axis=mybir.AxisListType.X)
                        nc.vector.tensor_tensor(out=stats[:, 0:1],
                                                in0=stats[:, 0:1],
                                                in1=red[:],
                                                op=mybir.AluOpType.add)
                        sq = sm.tile([F, P], F32, tag="sq")
                        nc.scalar.activation(
                            out=sq[:], in_=hsl,
                            func=mybir.ActivationFunctionType.Square)
                        nc.vector.reduce_sum(out=red[:], in_=sq[:],
                                             axis=mybir.AxisListType.X)
                        nc.vector.tensor_tensor(out=stats[:, 1:2],
                                                in0=stats[:, 1:2],
                                                in1=red[:],
                                                op=mybir.AluOpType.add)
                    else:
                        dmax = sm.tile([P, HEADS], F32, tag="dmax")
                        nc.vector.tensor_scalar_max(out=dmax[:],
                                                    in0=acc[:, F2:F2 + 4],
                                                    scalar1=1e-16)
                        rec = sm.tile([P, HEADS], F32, tag="rec")
                        nc.vector.reciprocal(out=rec[:], in_=dmax[:])
                        rst = sm.tile([P, F2], F32, tag="rst")
                        nc.vector.tensor_tensor(
                            out=rst[:].rearrange("p (h o) -> p h o", h=HEADS),
                            in0=acc[:, :F2].rearrange("p (h o) -> p h o",
                                                      h=HEADS),
                            in1=rec[:].unsqueeze(2).to_broadcast(
                                [P, HEADS, OUTC]),
                            op=mybir.AluOpType.mult)
                        res_ps = ps.tile([P, F2], F32, tag="aux")
                        nc.tensor.matmul(out=res_ps[:],
                                         lhsT=xT[:, b * P:(b + 1) * P],
                                         rhs=rW[:], start=True, stop=True)
                        nc.vector.tensor_tensor(out=rst[:], in0=rst[:],
                                                in1=res_ps[:],
                                                op=mybir.AluOpType.add)
                        nc.vector.tensor_tensor(out=rst[:], in0=rst[:],
                                                in1=b2bc[:],
                                                op=mybir.AluOpType.add)
                        nc.sync.dma_start(out=out[b * P:(b + 1) * P, :],
                                          in_=rst[:])

            # ---- phase C: BN + writeback ----
            if layer < 2:
                nc.sync.dma_start(out=stats_in[:], in_=stats[:])
                nc.gpsimd.collective_compute(
                    "AllReduce", mybir.AluOpType.add, replica_groups=rg,
                    ins=[stats_in[:].opt()], outs=[stats_out[:].opt()])
                tc.strict_bb_all_engine_barrier()
                ssb = cons.tile([F, 2], F32, tag="ssb")
                nc.sync.dma_start(out=ssb[:], in_=stats_out[:])
                mu = cons.tile([F, 1], F32, tag="mu")
                tmp1 = cons.tile([F, 1], F32, tag="tmp1")
                scl = cons.tile([F, 1], F32, tag="scl")
                bia = cons.tile([F, 1], F32, tag="bia")
                musq = cons.tile([F, 1], F32, tag="musq")
                invn = 1.0 / N
                nc.vector.tensor_scalar_mul(out=mu[:], in0=ssb[:, 0:1],
                                            scalar1=invn)
                nc.vector.tensor_scalar_mul(out=tmp1[:], in0=ssb[:, 1:2],
                                            scalar1=invn)
                nc.scalar.activation(out=musq[:], in_=mu[:],
                                     func=mybir.ActivationFunctionType.Square)
                nc.vector.tensor_tensor(out=tmp1[:], in0=tmp1[:], in1=musq[:],
                                        op=mybir.AluOpType.subtract)
                nc.vector.tensor_scalar_add(out=tmp1[:], in0=tmp1[:],
                                            scalar1=EPS)
                nc.scalar.activation(out=tmp1[:], in_=tmp1[:],
                                     func=mybir.ActivationFunctionType.Sqrt)
                nc.vector.reciprocal(out=tmp1[:], in_=tmp1[:])
                nc.vector.tensor_tensor(out=scl[:], in0=tmp1[:], in1=gam[:],
                                        op=mybir.AluOpType.mult)
                nc.vector.tensor_tensor(out=tmp1[:], in0=mu[:], in1=scl[:],
                                        op=mybir.AluOpType.mult)
                nc.vector.tensor_tensor(out=bia[:], in0=bet[:], in1=tmp1[:],
                                        op=mybir.AluOpType.subtract)
                nc.scalar.activation(out=yT[:, :], in_=yT[:, :],
                                     func=mybir.ActivationFunctionType.Relu,
                                     scale=scl[:, 0:1], bias=bia[:, 0:1])
                if NPC < NOWN:
                    nc.vector.memset(yT[:, NPC:NOWN], 0.0)
                nc.sync.dma_start(out=y_out[:], in_=yT[:])

    nc.compile()
    return nc


# ---------------------------------------------------------------- host glue

def make_in_maps(pr, inputs):
    feat = np.asarray(inputs["feat"], np.float32)
    W0 = np.asarray(inputs["W0"], np.float32)
    W1 = np.asarray(inputs["W1"], np.float32)
    W2 = np.asarray(inputs["W2"], np.float32)
    W0e = np.concatenate([W0, _fold(W0, np.asarray(inputs["al0"]), HID),
                          _fold(W0, np.asarray(inputs["ar0"]), HID)], axis=1)
    W1e = np.concatenate([W1, _fold(W1, np.asarray(inputs["al1"]), HID),
                          _fold(W1, np.asarray(inputs["ar1"]), HID)], axis=1)
    W2e = np.concatenate([W2, _fold(W2, np.asarray(inputs["al2"]), OUTC),
                          _fold(W2, np.asarray(inputs["ar2"]), OUTC)], axis=1)
    iota = np.tile(np.arange(P, dtype=np.float32)[None, :], (P, 1))
    common = {
        "iota": iota.astype(BF),
        "bsel": np.repeat(np.eye(HEADS, dtype=np.float32), HID, axis=1),
    }
    maps = [[], [], []]
    for r in range(NC):
        ids = np.nonzero(pr.core_of == r)[0]
        fp = np.zeros((NOWN, IN), np.float32)
        fp[pr.pos[ids]] = feat[ids]
        idx = {"idx16": pr.idx16[r], "colv": pr.colv[r], "qv": pr.qv[r],
               "minsl": pr.minsl[r]}
        maps[0].append({
            "x_in": np.ascontiguousarray(fp.T).astype(BF),
            "Wext": W0e.astype(BF),
            "resW": np.asarray(inputs["resW0"], np.float32).astype(BF),
            "gamma": np.asarray(inputs["gamma0"], np.float32).reshape(F, 1),
            "beta": np.asarray(inputs["beta0"], np.float32).reshape(F, 1),
            **idx, **common,
        })
        maps[1].append({
            "Wext": W1e.astype(BF),
            "gamma": np.asarray(inputs["gamma1"], np.float32).reshape(F, 1),
            "beta": np.asarray(inputs["beta1"], np.float32).reshape(F, 1),
            **idx, **common,
        })
        maps[2].append({
            "Wext": W2e.astype(BF),
            "resW": np.asarray(inputs["resW2"], np.float32).astype(BF),
            "b2bc": np.tile(np.asarray(inputs["b2"],
                                       np.float32).reshape(1, F2), (P, 1)),
            "iota": common["iota"],
            "idx16": pr.idx16[r], "colv": pr.colv[r], "qv": pr.qv[r],
            "minsl": pr.minsl[r],
        })
    return maps


_PROG_CACHE = {}


def get_program(pr, layer):
    key = layer
    if key not in _PROG_CACHE:
        _PROG_CACHE[key] = build_layer_program(pr, layer)
    return _PROG_CACHE[key]


def run(inputs, trace=False, trace_cores=None):
    pr = preprocess(np.asarray(inputs["src"]), np.asarray(inputs["dst"]))
    maps = make_in_maps(pr, inputs)
    cores = list(range(NC))
    total_ns = 0
    layer_res = []
    for layer in range(3):
        nc = get_program(pr, layer)
        res = bass_utils.run_bass_kernel_spmd(
            nc, maps[layer], core_ids=cores,
            trace=trace, trace_cores=trace_cores)
        layer_res.append(res)
        if res.exec_time_ns:
            total_ns += res.exec_time_ns
        if layer < 2:
            for r in range(NC):
                maps[layer + 1][r]["x_in"] = res.results[r]["y_out"]
    outp = np.empty((N, F2), np.float32)
    for r in range(NC):
        ids = np.nonzero(pr.core_of == r)[0]
        outp[ids] = layer_res[2].results[r]["out"][pr.pos[ids]]
    return outp, (total_ns, layer_res)


def kernel(**inputs) -> np.ndarray:
    return run(inputs)[0]


# revision 20
# speedup vs baseline: 1.0399x; 1.0399x over previous
"""3-layer GAT forward on 8 Trainium2 cores — v3 (dma_gather edition).

Structure (per layer):
  - Nodes sorted by in-degree and dealt round-robin to 8 cores, then packed
    into 98 blocks of 128 consecutive (uniform-degree) nodes per core; the
    node's slot is its position in the block. Degree-sorted dealing makes the
    per-(block, slab) tile structure near-identical across cores, so one SPMD
    program (padded to the max across cores) serves all 8.
  - Gather table rows are 256 B: [W@h bf16 x64 | el f32 x4 | pad]. Each core
    computes its own rows (phase A), AllGathers the table, then gathers
    per-edge source rows with dma_gather (int16 indices -> the table is
    addressed in 4 slabs of 2 cores, 25088 rows each). Edges are grouped by
    (block-group, slab); within a (block, slab) segment they are sorted by
    destination slot and split into 128 balanced chunks -> partition p's
    edges span only a few consecutive slots.
  - er (attention right-term) per edge: one plain indirect DMA per block
    reads, for each partition, k consecutive er rows starting at its chunk's
    min slot; a small one-hot (q = slot - minslot) selects per-edge er on DVE.
  - ee = exp(leaky_relu(el+er)); V = [ee*feat | ee] bf16; one-hot O over dst
    slots (bf16); per-tile matmul accumulates [sum(ee*feat) | sum(ee)] in
    PSUM; softmax divide per node afterwards. Layers 0/1 transposed
    ([channel, node]) with BN stats AllReduced; layer 2 node-major.
"""

import numpy as np
import ml_dtypes

import concourse.bass as bass
import concourse.bacc as bacc
import concourse.tile as tile
from concourse import mybir, bass_utils, library_config

F32 = mybir.dt.float32
I32 = mybir.dt.int32
I16 = mybir.dt.int16
I8 = mybir.dt.int8
BF16 = mybir.dt.bfloat16
BF = ml_dtypes.bfloat16

N, E, NC = 100000, 1600000, 8
IN, HID, HEADS, OUTC = 128, 16, 4, 40
F = HEADS * HID            # 64
F2 = HEADS * OUTC          # 160
P = 128
NPC = N // NC              # 12500
B = (NPC + P - 1) // P     # 98
NOWN = B * P               # 12544
NPAD = NC * NOWN
NSLAB = 4
SLABR = 2 * NOWN           # 25088 rows per index slab (< 2^15)
ELEM = 128                 # bf16 units per table row (256 B)
ELEM2 = 256                # layer-2 row (512 B)
NEG, EPS = 0.2, 1e-5
TB = 64                    # tile budget per block-group
GBMAX = 8                  # max blocks per group
SLABMAX = 48               # max tiles of one (group, slab)
KMAX = 64                  # max er slot-span


# ---------------------------------------------------------------- host prep

class Prep:
    pass


def preprocess(src, dst):
    """Graph preprocessing. Returns a Prep with global structure + per-core
    index arrays."""
    src = np.asarray(src, np.int64)
    dst = np.asarray(dst, np.int64)
    pr = Prep()
    deg = np.bincount(dst, minlength=N)
    order = np.argsort(-deg, kind="stable")
    core_of = np.empty(N, np.int64)
    pos = np.empty(N, np.int64)
    core_of[order] = np.arange(N) % NC
    pos[order] = np.arange(N) // NC
    pr.core_of, pr.pos = core_of, pos

    b_e = pos[dst] // P
    s_e = pos[dst] % P
    r_e = core_of[dst]
    c4_e = core_of[src] // 2
    srow_e = (core_of[src] % 2) * NOWN + pos[src]

    # per (core, block, slab) segment sizes
    key = (r_e * B + b_e) * NSLAB + c4_e
    cnt = np.bincount(key, minlength=NC * B * NSLAB).reshape(NC, B, NSLAB)
    S = np.ceil(cnt.max(axis=0) / P).astype(np.int64)      # [B, NSLAB]
    S = np.maximum(S, 1)
    Tb = S.sum(axis=1)                                      # tiles per block
    assert Tb.max() <= TB

    # block groups under tile budget TB (and <= GBMAX blocks)
    groups = []
    b0 = 0
    while b0 < B:
        b1, tot = b0, 0
        while b1 < B and b1 - b0 < GBMAX and tot + Tb[b1] <= TB:
            tot += Tb[b1]
            b1 += 1
        groups.append((b0, b1))
        b0 = b1
    pr.groups = groups
    pr.S, pr.Tb = S, Tb

    # column layouts: per (group, slab, block, tile)
    # tile-column order (for colv/q/G): g-major, then slab, then block, tile
    tile_off = {}      # (g, c4, b) -> tile col offset (global)
    ioff = {}          # (g, c4) -> idx16 column offset, n_idx
    goff = []          # group -> (tile col offset, total tiles)
    tcol = 0
    icol = 0
    for g, (ga, gb) in enumerate(groups):
        gt0 = tcol
        for c4 in range(NSLAB):
            n_idx = int(S[ga:gb, c4].sum()) * P
            ioff[(g, c4)] = (icol, n_idx)
            icol += n_idx // 16
            for b in range(ga, gb):
                tile_off[(g, c4, b)] = tcol
                tcol += int(S[b, c4])
        goff.append((gt0, tcol - gt0))
    TOT_TILES = tcol
    pr.tile_off, pr.ioff, pr.goff = tile_off, ioff, goff
    pr.TOT_TILES, pr.TOTC16 = TOT_TILES, icol

    # per-core arrays
    sortk = np.lexsort((s_e, key))
    eid_sorted = np.arange(E)[sortk]
    starts = np.concatenate([[0], np.cumsum(cnt.reshape(-1))])

    pr.idx16 = np.zeros((NC, 128, icol), np.int16)
    pr.colv = np.full((NC, P, TOT_TILES), -1, np.int8)
    pr.qv = np.full((NC, P, TOT_TILES), -1, np.int8)
    pr.minsl = np.zeros((NC, P, B), np.int32)
    kg = np.ones(len(groups), np.int64)

    for r in range(NC):
        mnsl = np.full((B, P), 10000, np.int64)
        mxsl = np.full((B, P), -1, np.int64)
        seg_cache = {}
        for b in range(B):
            for c4 in range(NSLAB):
                seg = (r * B + b) * NSLAB + c4
                e0, e1 = starts[seg], starts[seg + 1]
                eids = eid_sorted[e0:e1]          # slot-sorted
                n = e1 - e0
                base, rem = divmod(int(n), P)
                sizes = np.full(P, base, np.int64)
                sizes[:rem] += 1
                cstart = np.concatenate([[0], np.cumsum(sizes)])
                seg_cache[(b, c4)] = (eids, sizes, cstart)
                if n:
                    sl = s_e[eids]
                    pidx = np.searchsorted(cstart, np.arange(int(n)),
                                           side="right") - 1
                    np.minimum.at(mnsl[b], pidx, sl)
                    np.maximum.at(mxsl[b], pidx, sl)
        mnsl = np.where(mxsl >= 0, mnsl, 0)
        span = np.where(mxsl >= 0, mxsl - mnsl + 1, 1)
        pr.minsl[r] = (np.arange(B)[:, None] * P + mnsl).T.astype(np.int32)
        for g, (ga, gb) in enumerate(groups):
            kg[g] = max(kg[g], int(span[ga:gb].max()))
        for g, (ga, gb) in enumerate(groups):
            for c4 in range(NSLAB):
                ic0, n_idx = pr.ioff[(g, c4)]
                jt = 0   # tile index within this instruction
                for b in range(ga, gb):
                    eids, sizes, cstart = seg_cache[(b, c4)]
                    tc0 = pr.tile_off[(g, c4, b)]
                    sl = s_e[eids]
                    sr = srow_e[eids]
                    for t in range(int(S[b, c4])):
                        # partition p takes chunk p's t-th edge
                        pmask = sizes > t
                        ps = np.nonzero(pmask)[0]
                        ei = cstart[ps] + t
                        col = tc0 + t
                        pr.colv[r, ps, col] = sl[ei].astype(np.int8)
                        pr.qv[r, ps, col] = (sl[ei] - mnsl[b, ps]).astype(
                            np.int8)
                        # idx16 for j = (jt*128 + p)
                        j = (jt * P + ps)
                        v = sr[ei].astype(np.int16)
                        pr.idx16[r, (j % 16), ic0 + j // 16] = v
                        jt += 1
        # replicate idx rows across the 8 16-partition bands
        pr.idx16[r] = np.tile(pr.idx16[r, :16], (8, 1))
    pr.kg = [int(k) for k in kg]
    pr.KM = max(pr.kg)
    assert pr.KM <= KMAX, pr.kg
    pr.SM = max(int(S[ga:gb, c4].sum())
                for (ga, gb) in groups for c4 in range(NSLAB))
    assert pr.SM <= SLABMAX
    return pr


def _fold(W, a, dph):
    return np.einsum("khd,hd->kh", W.reshape(W.shape[0], HEADS, dph),
                     a).astype(np.float32)


# ---------------------------------------------------------------- program

def build_layer_program(pr, layer):
    fo = F2 if layer == 2 else F
    fin = IN if layer == 0 else F
    elem = ELEM2 if layer == 2 else ELEM
    fo4 = fo + 4
    nc = bacc.Bacc("TRN2", target_bir_lowering=False, debug=False,
                   num_devices=NC)
    S, groups, kg = pr.S, pr.groups, pr.kg

    def inp(name, shape, dt=F32):
        return nc.dram_tensor(name, shape, dt, kind="ExternalInput").ap()

    x_in = inp("x_in", [fin, NOWN], BF16)
    idx16_i = inp("idx16", [128, pr.TOTC16], I16)
    colv_i = inp("colv", [P, pr.TOT_TILES], I8)
    qv_i = inp("qv", [P, pr.TOT_TILES], I8)
    minsl_i = inp("minsl", [P, B], I32)
    Wext_i = inp("Wext", [fin, fo + 8], BF16)
    iota_i = inp("iota", [P, P], BF16)
    if layer < 2:
        gamma_i = inp("gamma", [F, 1])
        beta_i = inp("beta", [F, 1])
        bsel_i = inp("bsel", [HEADS, F])
        y_out = nc.dram_tensor("y_out", [F, NOWN], BF16,
                               kind="ExternalOutput").ap()
    if layer == 0:
        resW_i = inp("resW", [IN, F], BF16)
    if layer == 2:
        resW_i = inp("resW", [F, F2], BF16)
        b2bc_i = inp("b2bc", [P, F2])
        out = nc.dram_tensor("out", [NOWN, F2], F32,
                             kind="ExternalOutput").ap()

    rg = [list(range(NC))]

    with tile.TileContext(nc) as tc:
        with (
            tc.tile_pool(name="big", bufs=1) as bigp,
            tc.tile_pool(name="const", bufs=1) as cons,
            tc.tile_pool(name="gt", bufs=2) as gtp,
            tc.tile_pool(name="wk", bufs=2) as wk,
            tc.tile_pool(name="sm", bufs=3 if layer < 2 else 2) as sm,
            tc.tile_pool(name="ps", bufs=2, space="PSUM") as ps,
            tc.tile_pool(name="dram", bufs=1, space="DRAM") as dr,
        ):
            nc.gpsimd.load_library(library_config.mlp)

            tbl_own = dr.tile([NOWN, elem], BF16)
            tbl_full = dr.tile([NPAD, elem], BF16, addr_space="Shared")
            er_dram = dr.tile([NOWN + 2 * P, 4], F32)
            if layer < 2:
                stats_in = dr.tile([F, 2], F32)
                stats_out = dr.tile([F, 2], F32, addr_space="Shared")

            xT = bigp.tile([fin, NOWN], BF16, tag="xT")
            nc.sync.dma_start(out=xT[:], in_=x_in[:])
            if layer < 2:
                yT = bigp.tile([F, NOWN], BF16, tag="yT")
            colv_all = bigp.tile([P, pr.TOT_TILES], I8, tag="colv")
            qv_all = bigp.tile([P, pr.TOT_TILES], I8, tag="qv")
            minsl = bigp.tile([P, B], I32, tag="minsl")
            nc.sync.dma_start(out=colv_all[:], in_=colv_i[:])
            nc.sync.dma_start(out=qv_all[:], in_=qv_i[:])
            nc.sync.dma_start(out=minsl[:], in_=minsl_i[:])

            Wext = cons.tile([fin, fo + 8], BF16, tag="Wext")
            nc.sync.dma_start(out=Wext[:], in_=Wext_i[:])
            iota_b = cons.tile([P, P], BF16, tag="iota")
            nc.sync.dma_start(out=iota_b[:], in_=iota_i[:])
            if layer < 2:
                bsel = cons.tile([HEADS, F], F32, tag="bsel")
                gam = cons.tile([F, 1], F32, tag="gam")
                bet = cons.tile([F, 1], F32, tag="bet")
                nc.sync.dma_start(out=bsel[:], in_=bsel_i[:])
                nc.sync.dma_start(out=gam[:], in_=gamma_i[:])
                nc.sync.dma_start(out=bet[:], in_=beta_i[:])
                stats = cons.tile([F, 2], F32, tag="stats")
                nc.vector.memset(stats[:], 0.0)
            if layer == 0:
                rW = cons.tile([IN, F], BF16, tag="rW")
                nc.sync.dma_start(out=rW[:], in_=resW_i[:])
            if layer == 2:
                rW = cons.tile([F, F2], BF16, tag="rW")
                nc.sync.dma_start(out=rW[:], in_=resW_i[:])
                b2bc = cons.tile([P, F2], F32, tag="b2bc")
                nc.sync.dma_start(out=b2bc[:], in_=b2bc_i[:])
            zero4 = cons.tile([P, 4], F32, tag="zero4")
            nc.vector.memset(zero4[:], 0.0)

            # ---- phase A: table rows + er + AllGather ----
            CH = 7
            for c0 in range(0, B, CH):
                stg = wk.tile([P, CH * elem], BF16, tag="stg")
                stg_e = wk.tile([P, CH * 4], F32, tag="stg_e")
                nc.vector.memset(stg[:], 0.0)
                for ci in range(CH):
                    b = c0 + ci
                    tp = ps.tile([P, fo + 8], F32, tag="tp")
                    nc.tensor.matmul(out=tp[:], lhsT=xT[:, b * P:(b + 1) * P],
                                     rhs=Wext[:], start=True, stop=True)
                    nc.vector.tensor_copy(out=stg[:, ci * elem:ci * elem + fo],
                                          in_=tp[:, :fo])
                    nc.vector.tensor_copy(
                        out=stg[:, ci * elem + fo:ci * elem + fo + 8]
                        .bitcast(F32),
                        in_=tp[:, fo:fo + 4])
                    nc.vector.tensor_copy(out=stg_e[:, ci * 4:(ci + 1) * 4],
                                          in_=tp[:, fo + 4:fo + 8])
                nc.sync.dma_start(
                    out=tbl_own[c0 * P:(c0 + CH) * P, :].rearrange(
                        "(c p) f -> p c f", c=CH),
                    in_=stg[:].rearrange("p (c f) -> p c f", c=CH))
                nc.sync.dma_start(
                    out=er_dram[c0 * P:(c0 + CH) * P, :].rearrange(
                        "(c p) f -> p c f", c=CH),
                    in_=stg_e[:].rearrange("p (c f) -> p c f", c=CH))
            nc.sync.dma_start(out=er_dram[NOWN:NOWN + P, :], in_=zero4[:])
            nc.sync.dma_start(out=er_dram[NOWN + P:NOWN + 2 * P, :],
                              in_=zero4[:])
            nc.gpsimd.collective_compute(
                "AllGather", mybir.AluOpType.bypass, replica_groups=rg,
                ins=[tbl_own[:].opt()], outs=[tbl_full[:].opt()])
            tc.strict_bb_all_engine_barrier()

            # ---- phase B ----
            KM = pr.KM
            SMX = pr.SM
            for g, (ga, gb) in enumerate(groups):
                k = kg[g]
                nb = gb - ga
                gt0, gtn = pr.goff[g]
                ic_a = pr.ioff[(g, 0)][0]
                ic_b = pr.ioff[(g, 3)][0] + pr.ioff[(g, 3)][1] // 16
                idxt = wk.tile([128, TB * 8], I16, tag="idxt")
                nc.sync.dma_start(out=idxt[:, :ic_b - ic_a],
                                  in_=idx16_i[:, ic_a:ic_b])

                G = gtp.tile([P, TB * elem], BF16, tag="G")
                for c4 in range(NSLAB):
                    ic0, n_idx = pr.ioff[(g, c4)]
                    t0 = pr.tile_off[(g, c4, ga)] - gt0
                    nt = n_idx // P
                    # SWDGE ring holds ~1024 descriptors; larger single
                    # gathers deadlock the Q7 descriptor generator.
                    npc = (nt + 7) // 8
                    bq, rq = divmod(nt, npc) if npc else (0, 0)
                    qstarts = [0]
                    for i in range(npc):
                        qstarts.append(qstarts[-1] + bq + (1 if i < rq else 0))
                    for pi in range(npc):
                        q0, q1 = qstarts[pi], qstarts[pi + 1]
                        nq = q1 - q0
                        nc.gpsimd.dma_gather(
                            G[:, (t0 + q0) * elem:(t0 + q0 + nq) * elem]
                            .rearrange("p (t e) -> p t e", e=elem),
                            tbl_full[c4 * SLABR:(c4 + 1) * SLABR, :],
                            idxt[:, ic0 - ic_a + q0 * 8:
                                 ic0 - ic_a + (q0 + nq) * 8],
                            nq * P, nq * P, elem)

                # er rows per block: one [P, k*4] contiguous-run gather each
                erg = wk.tile([P, GBMAX * KM * 4], F32, tag="erg")
                for b in range(ga, gb):
                    br = b - ga
                    nc.gpsimd.indirect_dma_start(
                        out=erg[:, br * KM * 4:br * KM * 4 + k * 4],
                        out_offset=None, in_=er_dram[:],
                        in_offset=bass.IndirectOffsetOnAxis(
                            ap=minsl[:, b:b + 1], axis=0))
                ergb = wk.tile([P, GBMAX * KM * 4], BF16, tag="ergb")
                nc.vector.tensor_copy(out=ergb[:, :nb * KM * 4],
                                      in_=erg[:, :nb * KM * 4])

                V = gtp.tile([P, TB * fo4], BF16, tag="V")
                O = gtp.tile([P, TB * P], BF16, tag="O")
                for c4 in range(NSLAB):
                    sgc = int(S[ga:gb, c4].sum())
                    gcol = pr.tile_off[(g, c4, ga)]          # global col
                    t0 = gcol - gt0                          # in group tile
                    colb = sm.tile([P, SMX], BF16, tag="colb")
                    qb = sm.tile([P, SMX], BF16, tag="qb")
                    nc.vector.tensor_copy(
                        out=colb[:, :sgc], in_=colv_all[:, gcol:gcol + sgc])
                    nc.vector.tensor_copy(
                        out=qb[:, :sgc], in_=qv_all[:, gcol:gcol + sgc])
                    Q = sm.tile([P, SMX * KM], BF16, tag="Q")
                    Q3 = Q[:, :sgc * k].rearrange("p (s k) -> p s k", k=k)
                    nc.vector.tensor_tensor(
                        out=Q3,
                        in0=qb[:, :sgc].unsqueeze(2).to_broadcast(
                            [P, sgc, k]),
                        in1=iota_b[:, :k].unsqueeze(1).to_broadcast(
                            [P, sgc, k]),
                        op=mybir.AluOpType.is_equal)
                    tmp = sm.tile([P, SMX * 4 * KM], BF16, tag="tmp")
                    tmp4 = tmp[:, :sgc * 4 * k].rearrange(
                        "p (s h k) -> p s h k", h=4, k=k)
                    for b in range(ga, gb):
                        sb = int(S[b, c4])
                        br = b - ga
                        bo = pr.tile_off[(g, c4, b)] - gcol
                        nc.vector.tensor_tensor(
                            out=tmp4[:, bo:bo + sb],
                            in0=Q3[:, bo:bo + sb].unsqueeze(2).to_broadcast(
                                [P, sb, 4, k]),
                            in1=ergb[:, br * KM * 4:br * KM * 4 + k * 4]
                            .rearrange("p (k h) -> p h k", h=4).unsqueeze(1)
                            .to_broadcast([P, sb, 4, k]),
                            op=mybir.AluOpType.mult)
                    ere = sm.tile([P, SMX * 4], F32, tag="ere")
                    nc.vector.reduce_sum(
                        out=ere[:, :sgc * 4],
                        in_=tmp[:, :sgc * 4 * k].rearrange(
                            "p (x k) -> p x k", k=k),
                        axis=mybir.AxisListType.X)

                    G3 = G[:, t0 * elem:(t0 + sgc) * elem].rearrange(
                        "p (t e) -> p t e", e=elem)
                    el_view = G3[:, :, fo:fo + 8].bitcast(F32)
                    ee = sm.tile([P, SMX * 4], F32, tag="ee")
                    ee3 = ee[:, :sgc * 4].rearrange("p (s h) -> p s h", h=4)
                    nc.vector.tensor_tensor(
                        out=ee3, in0=el_view,
                        in1=ere[:, :sgc * 4].rearrange("p (s h) -> p s h",
                                                       h=4),
                        op=mybir.AluOpType.add)
                    nc.vector.scalar_tensor_tensor(
                        out=ee[:, :sgc * 4], in0=ee[:, :sgc * 4], scalar=NEG,
                        in1=ee[:, :sgc * 4], op0=mybir.AluOpType.mult,
                        op1=mybir.AluOpType.max)
                    eeb = sm.tile([P, SMX * 4], BF16, tag="eeb")
                    nc.scalar.activation(out=eeb[:, :sgc * 4],
                                         in_=ee[:, :sgc * 4],
                                         func=mybir.ActivationFunctionType.Exp)
                    eeb3 = eeb[:, :sgc * 4].rearrange("p (s h) -> p s h", h=4)

                    V3 = V[:, t0 * fo4:(t0 + sgc) * fo4].rearrange(
                        "p (s f) -> p s f", f=fo4)
                    dph = OUTC if layer == 2 else HID
                    nc.vector.tensor_tensor(
                        out=V3[:, :, :fo].rearrange("p s (h d) -> p s h d",
                                                    h=HEADS),
                        in0=G3[:, :, :fo].rearrange("p s (h d) -> p s h d",
                                                    h=HEADS),
                        in1=eeb3.unsqueeze(3).to_broadcast(
                            [P, sgc, HEADS, dph]),
                        op=mybir.AluOpType.mult)
                    nc.vector.tensor_copy(out=V3[:, :, fo:fo4], in_=eeb3)

                    O3 = O[:, t0 * P:(t0 + sgc) * P].rearrange(
                        "p (s c) -> p s c", c=P)
                    nc.vector.tensor_tensor(
                        out=O3,
                        in0=iota_b[:].unsqueeze(1).to_broadcast([P, sgc, P]),
                        in1=colb[:, :sgc].unsqueeze(2).to_broadcast(
                            [P, sgc, P]),
                        op=mybir.AluOpType.is_equal)

                # matmuls + per-node epilogue, block-major
                for b in range(ga, gb):
                    ntile = int(pr.Tb[b])
                    if layer < 2:
                        acc = ps.tile([fo4, P], F32, tag="acc")
                    else:
                        acc = ps.tile([P, fo4], F32, tag="acc")
                    it = 0
                    for c4 in range(NSLAB):
                        for t in range(int(S[b, c4])):
                            j = pr.tile_off[(g, c4, b)] - gt0 + t
                            if layer < 2:
                                nc.tensor.matmul(
                                    out=acc[:],
                                    lhsT=V[:, j * fo4:(j + 1) * fo4],
                                    rhs=O[:, j * P:(j + 1) * P],
                                    start=(it == 0), stop=(it == ntile - 1))
                            else:
                                nc.tensor.matmul(
                                    out=acc[:],
                                    lhsT=O[:, j * P:(j + 1) * P],
                                    rhs=V[:, j * fo4:(j + 1) * fo4],
                                    start=(it == 0), stop=(it == ntile - 1))
                            it += 1

                    if layer < 2:
                        dmax = sm.tile([HEADS, P], F32, tag="dmax")
                        nc.vector.tensor_scalar_max(out=dmax[:],
                                                    in0=acc[F:F + 4, :],
                                                    scalar1=1e-16)
                        rec = sm.tile([HEADS, P], F32, tag="rec")
                        nc.vector.reciprocal(out=rec[:], in_=dmax[:])
                        recb_ps = ps.tile([F, P], F32, tag="aux")
                        nc.tensor.matmul(out=recb_ps[:], lhsT=bsel[:],
                                         rhs=rec[:], start=True, stop=True)
                        recb = sm.tile([F, P], F32, tag="recb")
                        nc.vector.tensor_copy(out=recb[:], in_=recb_ps[:])
                        hsl = yT[:, b * P:(b + 1) * P]
                        hf = sm.tile([F, P], F32, tag="hf")
                        nc.vector.tensor_tensor(out=hf[:], in0=acc[:F, :],
                                                in1=recb[:],
                                                op=mybir.AluOpType.mult)
                        if layer == 0:
                            res_ps = ps.tile([F, P], F32, tag="aux")
                            nc.tensor.matmul(out=res_ps[:], lhsT=rW[:],
                                             rhs=xT[:, b * P:(b + 1) * P],
                                             start=True, stop=True)
                            nc.vector.tensor_tensor(out=hsl, in0=hf[:],
                                                    in1=res_ps[:],
                                                    op=mybir.AluOpType.add)
                        else:
                            hold = xT[:, b * P:(b + 1) * P]
                            nc.vector.tensor_tensor(out=hf[:], in0=hf[:],
                                                    in1=hold,
                                                    op=mybir.AluOpType.add)
                            nc.vector.tensor_tensor(out=hsl, in0=hf[:],
                                                    in1=hold,
                                                    op=mybir.AluOpType.add)
                        red = sm.tile([F, 1], F32, tag="red")
                        nc.vector.reduce_sum(out=red[:], in_=hsl,
                                             axis=mybir.AxisListType.X)
                        nc.vector.tensor_tensor(out=stats[:, 0:1],
                                                in0=stats[:, 0:1],
                                                in1=red[:],
                                                op=mybir.AluOpType.add)
                        sq = sm.tile([F, P], F32, tag="sq")
                        nc.scalar.activation(
                            out=sq[:], in_=hsl,
                            func=mybir.ActivationFunctionType.Square)
                        nc.vector.reduce_sum(out=red[:], in_=sq[:],
                                             axis=mybir.AxisListType.X)
                        nc.vector.tensor_tensor(out=stats[:, 1:2],
                                                in0=stats[:, 1:2],
                                                in1=red[:],
                                                op=mybir.AluOpType.add)
                    else:
                        dmax = sm.tile([P, HEADS], F32, tag="dmax")
                        nc.vector.tensor_scalar_max(out=dmax[:],
                                                    in0=acc[:, F2:F2 + 4],
                                                    scalar1=1e-16)
                        rec = sm.tile([P, HEADS], F32, tag="rec")
                        nc.vector.reciprocal(out=rec[:], in_=dmax[:])
                        rst = sm.tile([P, F2], F32, tag="rst")
                        nc.vector.tensor_tensor(
                            out=rst[:].rearrange("p (h o) -> p h o", h=HEADS),
                            in0=acc[:, :F2].rearrange("p (h o) -> p h o",
                                                      h=HEADS),
                            in1=rec[:].unsqueeze(2).to_broadcast(
                                [P, HEADS, OUTC]),
                            op=mybir.AluOpType.mult)
                        res_ps = ps.tile([P, F2], F32, tag="aux")
                        nc.tensor.matmul(out=res_ps[:],
                                         lhsT=xT[:, b * P:(b + 1) * P],
                                         rhs=rW[:], start=True, stop=True)
                        nc.vector.tensor_tensor(out=rst[:], in0=rst[:],
                                                in1=res_ps[:],
                                                op=mybir.AluOpType.add)
                        nc.vector.tensor_tensor(out=rst[:], in0=rst[:],
                                                in1=b2bc[:],
                                                op=mybir.AluOpType.add)
                        nc.sync.dma_start(out=out[b * P:(b + 1) * P, :],
                                          in_=rst[:])

            # ---- phase C: BN + writeback ----
            if layer < 2:
                nc.sync.dma_start(out=stats_in[:], in_=stats[:])
                nc.gpsimd.collective_compute(
                    "AllReduce", mybir.AluOpType.add, replica_groups=rg,
                    ins=[stats_in[:].opt()], outs=[stats_out[:].opt()])
                tc.strict_bb_all_engine_barrier()
                ssb = cons.tile([F, 2], F32, tag="ssb")
                nc.sync.dma_start(out=ssb[:], in_=stats_out[:])
                mu = cons.tile([F, 1], F32, tag="mu")
                tmp1 = cons.tile([F, 1], F32, tag="tmp1")
                scl = cons.tile([F, 1], F32, tag="scl")
                bia = cons.tile([F, 1], F32, tag="bia")
                musq = cons.tile([F, 1], F32, tag="musq")
                invn = 1.0 / N
                nc.vector.tensor_scalar_mul(out=mu[:], in0=ssb[:, 0:1],
                                            scalar1=invn)
                nc.vector.tensor_scalar_mul(out=tmp1[:], in0=ssb[:, 1:2],
                                            scalar1=invn)
                nc.scalar.activation(out=musq[:], in_=mu[:],
                                     func=mybir.ActivationFunctionType.Square)
                nc.vector.tensor_tensor(out=tmp1[:], in0=tmp1[:], in1=musq[:],
                                        op=mybir.AluOpType.subtract)
                nc.vector.tensor_scalar_add(out=tmp1[:], in0=tmp1[:],
                                            scalar1=EPS)
                nc.scalar.activation(out=tmp1[:], in_=tmp1[:],
                                     func=mybir.ActivationFunctionType.Sqrt)
                nc.vector.reciprocal(out=tmp1[:], in_=tmp1[:])
                nc.vector.tensor_tensor(out=scl[:], in0=tmp1[:], in1=gam[:],
                                        op=mybir.AluOpType.mult)
                nc.vector.tensor_tensor(out=tmp1[:], in0=mu[:], in1=scl[:],
                                        op=mybir.AluOpType.mult)
                nc.vector.tensor_tensor(out=bia[:], in0=bet[:], in1=tmp1[:],
                                        op=mybir.AluOpType.subtract)
                nc.scalar.activation(out=yT[:, :], in_=yT[:, :],
                                     func=mybir.ActivationFunctionType.Relu,
                                     scale=scl[:, 0:1], bias=bia[:, 0:1])
                if NPC < NOWN:
                    nc.vector.memset(yT[:, NPC:NOWN], 0.0)
                nc.sync.dma_start(out=y_out[:], in_=yT[:])

    nc.compile()
    return nc


# ---------------------------------------------------------------- host glue

def make_in_maps(pr, inputs):
    feat = np.asarray(inputs["feat"], np.float32)
    W0 = np.asarray(inputs["W0"], np.float32)
    W1 = np.asarray(inputs["W1"], np.float32)
    W2 = np.asarray(inputs["W2"], np.float32)
    W0e = np.concatenate([W0, _fold(W0, np.asarray(inputs["al0"]), HID),
                          _fold(W0, np.asarray(inputs["ar0"]), HID)], axis=1)
    W1e = np.concatenate([W1, _fold(W1, np.asarray(inputs["al1"]), HID),
                          _fold(W1, np.asarray(inputs["ar1"]), HID)], axis=1)
    W2e = np.concatenate([W2, _fold(W2, np.asarray(inputs["al2"]), OUTC),
                          _fold(W2, np.asarray(inputs["ar2"]), OUTC)], axis=1)
    iota = np.tile(np.arange(P, dtype=np.float32)[None, :], (P, 1))
    common = {
        "iota": iota.astype(BF),
        "bsel": np.repeat(np.eye(HEADS, dtype=np.float32), HID, axis=1),
    }
    maps = [[], [], []]
    for r in range(NC):
        ids = np.nonzero(pr.core_of == r)[0]
        fp = np.zeros((NOWN, IN), np.float32)
        fp[pr.pos[ids]] = feat[ids]
        idx = {"idx16": pr.idx16[r], "colv": pr.colv[r], "qv": pr.qv[r],
               "minsl": pr.minsl[r]}
        maps[0].append({
            "x_in": np.ascontiguousarray(fp.T).astype(BF),
            "Wext": W0e.astype(BF),
            "resW": np.asarray(inputs["resW0"], np.float32).astype(BF),
            "gamma": np.asarray(inputs["gamma0"], np.float32).reshape(F, 1),
            "beta": np.asarray(inputs["beta0"], np.float32).reshape(F, 1),
            **idx, **common,
        })
        maps[1].append({
            "Wext": W1e.astype(BF),
            "gamma": np.asarray(inputs["gamma1"], np.float32).reshape(F, 1),
            "beta": np.asarray(inputs["beta1"], np.float32).reshape(F, 1),
            **idx, **common,
        })
        maps[2].append({
            "Wext": W2e.astype(BF),
            "resW": np.asarray(inputs["resW2"], np.float32).astype(BF),
            "b2bc": np.tile(np.asarray(inputs["b2"],
                                       np.float32).reshape(1, F2), (P, 1)),
            "iota": common["iota"],
            "idx16": pr.idx16[r], "colv": pr.colv[r], "qv": pr.qv[r],
            "minsl": pr.minsl[r],
        })
    return maps


_PROG_CACHE = {}


def get_program(pr, layer):
    key = layer
    if key not in _PROG_CACHE:
        _PROG_CACHE[key] = build_layer_program(pr, layer)
    return _PROG_CACHE[key]


def run(inputs, trace=False, trace_cores=None):
    pr = preprocess(np.asarray(inputs["src"]), np.asarray(inputs["dst"]))
    maps = make_in_maps(pr, inputs)
    cores = list(range(NC))
    total_ns = 0
    layer_res = []
    for layer in range(3):
        nc = get_program(pr, layer)
        res = bass_utils.run_bass_kernel_spmd(
            nc, maps[layer], core_ids=cores,
            trace=trace, trace_cores=trace_cores)
        layer_res.append(res)
        if res.exec_time_ns:
            total_ns += res.exec_time_ns
        if layer < 2:
            for r in range(NC):
                maps[layer + 1][r]["x_in"] = res.results[r]["y_out"]
    outp = np.empty((N, F2), np.float32)
    for r in range(NC):
        ids = np.nonzero(pr.core_of == r)[0]
        outp[ids] = layer_res[2].results[r]["out"][pr.pos[ids]]
    return outp, (total_ns, layer_res)


def kernel(**inputs) -> np.ndarray:
    return run(inputs)[0]
